# revision 19
# baseline (speedup 1.0000x reference)
"""ANOVA-kernel (order 3) Trainium2 Bass kernel, v3 (fp16 + custom DVE op).

Math: per batch b, y[b] = sum_d e3(x[b, :, d]) with e3 the 3rd elementary
symmetric polynomial over F=64 fields. Newton's identities give

    e3 = (p1^3 - 3 p1 p2 + 2 p3) / 6,    p_k[b, d] = sum_f x[b, f, d]^k

so only grouped (per-d) p1/p2 reductions plus a full-row p3 sum are needed.

Engine plan per [128 x 4096] fp16 tile (batch on partitions, free =
(h, d, f2) with h = f//32 major so both f-halves are contiguous):

  - p3: Scalar engine evaluates sin(x/8) with a free per-partition
    accumulate; sum sin(t x) = t P1 - t^3 P3/6 + O(t^5), so
    P3 = 384 P1f - 3072 S1 (truncation ~2.4e-4 norm-rel).
  - p1 head: one DVE tensor_add folds the f-halves (fp16 2x mode) into a
    persistent wide buffer.
  - p2 head: custom DVE op SQSUM (sq(a)+sq(b)) fuses the square with the
    same fold in one port-optimal pass; a few tiles use ACT Square + DVE
    add instead to balance ACT/DVE.
  - tails: the remaining f 32->1 reduction is batched across 4-tile groups
    as wide in-place fp16 2x tensor_adds (one fp32 final level), which
    amortizes instruction/semaphore overhead. No GPSIMD: its SBUF port is
    shared with the DVE and concurrent tensor ops serialize both engines.
  - epilogue recombines in two halves to hide the tail.

Inputs are downcast to fp16 on the host (norm-rel error ~2e-3, far under
the 2e-2 gate), halving HBM traffic.

Sharding: pure data parallel over batch across 8 NeuronCores.
"""

import numpy as np

_B, _F, _D = 8192, 64, 64
_NCORES = 8
_BP = _B // _NCORES     # batches per core
_P = 128                # partitions per tile
_FD = _F * _D           # free elems per batch
_H = _FD // 2

# tiles whose p2 head uses ACT Square + DVE tt-add instead of the custom op
_ACT_SQ_TILES = 4

_SQSUM = [None]


def _get_sqsum_op():
    """Register the custom DVE op sq(Src0)+sq(Src1) at runtime (the
    documented dve_ops.OPS extension pattern; the uop table is generated
    per-NEFF at compile time)."""
    if _SQSUM[0] is not None:
        return _SQSUM[0]
    from concourse import dve_ops
    from concourse.dve_spec import Spec, Src0, Src1, lower, sq
    from concourse.dve_uop import DveOpSpec

    name = "ANOVA_SQSUM"
    for op in dve_ops.OPS:
        if op.name == name:
            _SQSUM[0] = op
            return op

    def _ref(in0, in1, s0, s1, imm2):
        return in0.astype(np.float32) ** 2 + in1.astype(np.float32) ** 2

    spec = Spec(body=sq(Src0) + sq(Src1), reference=_ref)
    row = 1 + len(dve_ops.OPS)
    shas = {}
    for ver in ("v3", "v4"):
        uops = lower(spec, ver=ver)
        shas[ver] = DveOpSpec(name=name, opcode=row, uops=uops, rd1_en=True).sha(ver)
    op = dve_ops.DveOp(name, spec, subdim=False, uops_sha=shas)
    dve_ops.OPS.append(op)
    dve_ops.CUSTOM_DVE_SPECS[name] = spec
    dve_ops._SUB_OPCODE_FOR_NAME[name] = row
    _SQSUM[0] = op
    return op


def build_nc(bp=_BP, act_sq_tiles=_ACT_SQ_TILES):
    """Per-core Bass graph.

    Inputs:  "x"   [bp, 4096] fp16, free layout (h=f//32, d, f%32)
    Outputs: "out" [128, bp/128] f32, out[p, t] = y[t*128 + p]
    """
    from contextlib import ExitStack

    from concourse import bacc, mybir, tile

    f16 = mybir.dt.float16
    f32 = mybir.dt.float32
    AF = mybir.ActivationFunctionType
    OP = mybir.AluOpType
    AX = mybir.AxisListType

    sqsum_op = _get_sqsum_op()

    T = bp // _P
    assert bp % _P == 0 and T % 2 == 0
    # spread ACT-square tiles over the stream (not the ramp tile)
    act_sq = set([2, 4, 6, 1, 5, 7][:act_sq_tiles]).intersection(range(T))

    nc = bacc.Bacc("TRN2", target_bir_lowering=False, debug=False)
    x_ext = nc.dram_tensor("x", [bp, _FD], f16, kind="ExternalInput").ap()
    y_ext = nc.dram_tensor("out", [_P, T], f32, kind="ExternalOutput").ap()

    with tile.TileContext(nc) as tc, ExitStack() as ctx:
        xp = ctx.enter_context(tc.tile_pool(name="x", bufs=4))
        sp = ctx.enter_context(tc.tile_pool(name="scr", bufs=3))
        pers = ctx.enter_context(tc.tile_pool(name="pers", bufs=1))

        # wide per-tile L1 outputs: [t, d, 32] fp16
        s1w = pers.tile([_P, T * 2048], f16, tag="s1w")
        s2w = pers.tile([_P, T * 2048], f16, tag="s2w")
        p1b = pers.tile([_P, T * _D], f32, tag="p1b")
        p2b = pers.tile([_P, T * _D], f32, tag="p2b")
        sa1 = pers.tile([_P, T + 2], f32, tag="sa1")   # sin accum (cols T, T+1: tile0 pieces)
        eacc = pers.tile([_P, T], f32, tag="eacc")
        p1f = pers.tile([_P, T], f32, tag="p1f")
        dq = pers.tile([_P, T], f32, tag="dq")
        out8 = pers.tile([_P, T], f32, tag="out8")
        er = pers.tile([_P, T * _D], f32, tag="er")
        ez = pers.tile([_P, T * _D], f32, tag="ez")

        xv_dram = x_ext.rearrange("(t p) q -> t p q", p=_P)

        # warm the Sin table during the first DMA wait (lazy load is ~2.6us)
        warm = pers.tile([_P, 1], f32, tag="warm")
        nc.gpsimd.memset(warm[:], 0.0)
        nc.scalar.activation(warm[:], warm[:], AF.Sin, scale=0.125)

        def emit_heads(k, xt, nd, sacol, scol):
            """Heads for a tile buffer xt holding nd d-groups (free nd*64,
            f-halves contiguous). L1 outputs land at s1w/s2w[:, scol:]."""
            fd = nd * _F
            h = fd // 2
            xlo = xt[:, :h]
            xhi = xt[:, h:fd]
            sscr = sp.tile([_P, _FD], f16, tag="sscr")
            nc.scalar.activation(
                sscr[:, :fd], xt[:, :fd], AF.Sin, scale=0.125,
                accum_out=sa1[:, sacol:sacol + 1],
            )
            nc.vector.tensor_add(s1w[:, scol:scol + h], xlo, xhi)
            if k in act_sq:
                x2t = sp.tile([_P, _FD], f16, tag="x2")
                nc.scalar.activation(x2t[:, :fd], xt[:, :fd], AF.Square)
                nc.vector.tensor_add(
                    s2w[:, scol:scol + h], x2t[:, :h], x2t[:, h:fd]
                )
            else:
                nc.vector._custom_dve(
                    sqsum_op, out=s2w[:, scol:scol + h], in0=xlo, in1=xhi
                )

        def emit_tails(t0, t1):
            """Batched fold 32->1 for tiles [t0, t1): wide in-place fp16 2x
            adds + one fp32 final level into p1b/p2b."""
            nt = t1 - t0
            for w, dst in ((s1w, p1b), (s2w, p2b)):
                v = w[:, t0 * 2048:t1 * 2048].rearrange(
                    "p (g f) -> p g f", f=32
                )  # g = nt*64 groups
                nc.vector.tensor_add(v[:, :, :16], v[:, :, :16], v[:, :, 16:])
                nc.vector.tensor_add(v[:, :, :8], v[:, :, :8], v[:, :, 8:16])
                nc.vector.tensor_add(v[:, :, :4], v[:, :, :4], v[:, :, 4:8])
                nc.vector.tensor_add(v[:, :, :2], v[:, :, :2], v[:, :, 2:4])
                nc.vector.tensor_add(
                    dst[:, t0 * _D:t1 * _D].rearrange("p (g o) -> p g o", o=1),
                    v[:, :, 0:1], v[:, :, 1:2],
                )

        def epilogue(c0, c1):
            """Combine p1/p2/sa1 for tile-columns [c0, c1)."""
            d0, d1 = c0 * _D, c1 * _D
            nc.vector.tensor_mul(er[:, d0:d1], p1b[:, d0:d1], p1b[:, d0:d1])
            nc.vector.scalar_tensor_tensor(
                ez[:, d0:d1], p2b[:, d0:d1], 3.0, er[:, d0:d1],
                OP.mult, OP.subtract,
            )
            nc.vector.scalar_tensor_tensor(
                er[:, d0:d1], p1b[:, d0:d1], -1.0 / 6.0, ez[:, d0:d1],
                OP.mult, OP.mult,
            )
            nc.vector.reduce_sum(
                eacc[:, c0:c1],
                er[:, d0:d1].rearrange("p (t d) -> p t d", t=c1 - c0, d=_D),
                axis=AX.X,
            )
            nc.vector.reduce_sum(
                p1f[:, c0:c1],
                p1b[:, d0:d1].rearrange("p (t d) -> p t d", t=c1 - c0, d=_D),
                axis=AX.X,
            )
            # out = eacc + 128 p1f - 1024 S1   (P3 = 384 p1f - 3072 S1)
            nc.vector.scalar_tensor_tensor(
                dq[:, c0:c1], sa1[:, c0:c1], -1024.0, eacc[:, c0:c1],
                OP.mult, OP.add,
            )
            nc.vector.scalar_tensor_tensor(
                out8[:, c0:c1], p1f[:, c0:c1], 128.0, dq[:, c0:c1],
                OP.mult, OP.add,
            )

        for k in range(T):
            if k == 0:
                # two half-tiles so compute starts ~2x sooner; the host
                # layout makes each half self-contained (d-split).
                xta = sp.tile([_P, _H], f16, tag="xta")
                nc.sync.dma_start(xta[:, :_H // 2], xv_dram[0][:, :_H // 2])
                nc.sync.dma_start(xta[:, _H // 2:], xv_dram[0][:, _H // 2:_H])
                emit_heads(0, xta[:, :_H // 2], _D // 4, 0, 0)
                emit_heads(0, xta[:, _H // 2:], _D // 4, T, 512)
                xtb = sp.tile([_P, _H], f16, tag="xtb")
                nc.sync.dma_start(xtb[:], xv_dram[0][:, _H:])
                emit_heads(0, xtb, _D // 2, T + 1, 1024)
                nc.vector.scalar_tensor_tensor(
                    sa1[:, 0:1], sa1[:, T:T + 1], 1.0, sa1[:, 0:1],
                    OP.mult, OP.add,
                )
                nc.vector.scalar_tensor_tensor(
                    sa1[:, 0:1], sa1[:, T + 1:T + 2], 1.0, sa1[:, 0:1],
                    OP.mult, OP.add,
                )
            else:
                xt = xp.tile([_P, _FD], f16, tag="xt")
                nc.sync.dma_start(xt[:], xv_dram[k])
                emit_heads(k, xt, _D, k, k * 2048)
            if k == T // 2:
                emit_tails(0, T // 2)
            if k == T // 2 + 1:
                epilogue(0, T // 2)
                nc.sync.dma_start(y_ext[:, :T // 2], out8[:, :T // 2])
            if k == T - 1:
                emit_tails(T // 2, T - 1)
        emit_tails(T - 1, T)
        epilogue(T // 2, T)
        nc.sync.dma_start(y_ext[:, T // 2:], out8[:, T // 2:])

    nc.compile()
    return nc


_nc_cache = {}


def _get_nc():
    key = (_BP, _ACT_SQ_TILES)
    if key not in _nc_cache:
        _nc_cache[key] = build_nc(*key)
    return _nc_cache[key]


def _marshal(x: np.ndarray) -> list:
    """FULL fp32 input [B, F, D] -> per-core fp16 arrays [bp, 4096] in
    (tile-internal) layout: per batch, free = (h=f//32, d, f%32), with
    tile 0 of each core d-split into two self-contained halves."""
    x = np.asarray(x)
    assert x.shape == (_B, _F, _D), x.shape
    xc = x.reshape(_NCORES, _BP, _F, _D).astype(np.float16)
    xt = xc.reshape(_NCORES, _BP, 2, 32, _D).transpose(0, 1, 2, 4, 3)
    out = np.empty((_NCORES, _BP, _FD), dtype=np.float16)
    flat = xt.reshape(_NCORES, _BP, _FD)
    out[:, _P:] = flat[:, _P:]
    # tile 0 pieces: d 0-15, d 16-31 (quarters), d 32-63 (half); each piece
    # h-major so its f-halves are contiguous and self-contained
    t0 = xt[:, :_P]                                   # [c, 128, h, d, f2]
    pa = t0[:, :, :, 0:16].reshape(_NCORES, _P, 1024)
    pb = t0[:, :, :, 16:32].reshape(_NCORES, _P, 1024)
    pc = t0[:, :, :, 32:64].reshape(_NCORES, _P, 2048)
    out[:, :_P] = np.concatenate([pa, pb, pc], axis=2)
    return [np.ascontiguousarray(out[c]) for c in range(_NCORES)]


def kernel(x: np.ndarray) -> np.ndarray:
    from concourse.bass_utils import run_bass_kernel_spmd

    nc = _get_nc()
    shards = _marshal(x)
    in_maps = [{"x": shards[c]} for c in range(_NCORES)]
    res = run_bass_kernel_spmd(nc, in_maps, core_ids=list(range(_NCORES)))
    outs = []
    for c in range(_NCORES):
        o = res.results[c]["out"]  # [128, T]; o[p, t] = y[t*128 + p]
        outs.append(np.asarray(o).T.reshape(-1))
    return np.concatenate(outs).reshape(_B, 1).astype(np.float32)


# revision 21
# speedup vs baseline: 1.0297x; 1.0297x over previous
"""ANOVA-kernel (order 3) Trainium2 Bass kernel, v3 (fp16 + custom DVE op).

Math: per batch b, y[b] = sum_d e3(x[b, :, d]) with e3 the 3rd elementary
symmetric polynomial over F=64 fields. Newton's identities give

    e3 = (p1^3 - 3 p1 p2 + 2 p3) / 6,    p_k[b, d] = sum_f x[b, f, d]^k

so only grouped (per-d) p1/p2 reductions plus a full-row p3 sum are needed.

Engine plan per [128 x 4096] fp16 tile (batch on partitions, free =
(h, d, f2) with h = f//32 major so both f-halves are contiguous):

  - p3: Scalar engine evaluates sin(x/8) with a free per-partition
    accumulate; sum sin(t x) = t P1 - t^3 P3/6 + O(t^5), so
    P3 = 384 P1f - 3072 S1 (truncation ~2.4e-4 norm-rel).
  - p1 head: one DVE tensor_add folds the f-halves (fp16 2x mode) into a
    persistent wide buffer.
  - p2 head: custom DVE op SQSUM (sq(a)+sq(b)) fuses the square with the
    same fold in one port-optimal pass; a few tiles use ACT Square + DVE
    add instead to balance ACT/DVE.
  - tails: the remaining f 32->1 reduction is batched across 4-tile groups
    as wide in-place fp16 2x tensor_adds (one fp32 final level), which
    amortizes instruction/semaphore overhead. No GPSIMD: its SBUF port is
    shared with the DVE and concurrent tensor ops serialize both engines.
  - epilogue recombines in two halves to hide the tail.

Inputs are downcast to fp16 on the host (norm-rel error ~2e-3, far under
the 2e-2 gate), halving HBM traffic.

Sharding: pure data parallel over batch across 8 NeuronCores.
"""

import numpy as np

_B, _F, _D = 8192, 64, 64
_NCORES = 8
_BP = _B // _NCORES     # batches per core
_P = 128                # partitions per tile
_FD = _F * _D           # free elems per batch
_H = _FD // 2

# tiles whose p2 head uses ACT Square + DVE tt-add instead of the custom op
_ACT_SQ_TILES = 3

_SQSUM = [None]


def _get_sqsum_op():
    """Register the custom DVE op sq(Src0)+sq(Src1) at runtime (the
    documented dve_ops.OPS extension pattern; the uop table is generated
    per-NEFF at compile time)."""
    if _SQSUM[0] is not None:
        return _SQSUM[0]
    from concourse import dve_ops
    from concourse.dve_spec import Spec, Src0, Src1, lower, sq
    from concourse.dve_uop import DveOpSpec

    name = "ANOVA_SQSUM"
    for op in dve_ops.OPS:
        if op.name == name:
            _SQSUM[0] = op
            return op

    def _ref(in0, in1, s0, s1, imm2):
        return in0.astype(np.float32) ** 2 + in1.astype(np.float32) ** 2

    spec = Spec(body=sq(Src0) + sq(Src1), reference=_ref)
    row = 1 + len(dve_ops.OPS)
    shas = {}
    for ver in ("v3", "v4"):
        uops = lower(spec, ver=ver)
        shas[ver] = DveOpSpec(name=name, opcode=row, uops=uops, rd1_en=True).sha(ver)
    op = dve_ops.DveOp(name, spec, subdim=False, uops_sha=shas)
    dve_ops.OPS.append(op)
    dve_ops.CUSTOM_DVE_SPECS[name] = spec
    dve_ops._SUB_OPCODE_FOR_NAME[name] = row
    _SQSUM[0] = op
    return op


def build_nc(bp=_BP, act_sq_tiles=_ACT_SQ_TILES):
    """Per-core Bass graph.

    Inputs:  "x"   [bp, 4096] fp16, free layout (h=f//32, d, f%32)
    Outputs: "out" [128, bp/128] f32, out[p, t] = y[t*128 + p]
    """
    from contextlib import ExitStack

    from concourse import bacc, mybir, tile

    f16 = mybir.dt.float16
    f32 = mybir.dt.float32
    AF = mybir.ActivationFunctionType
    OP = mybir.AluOpType
    AX = mybir.AxisListType

    sqsum_op = _get_sqsum_op()

    T = bp // _P
    assert bp % _P == 0 and T % 2 == 0
    # spread ACT-square tiles over the stream (not the ramp tile)
    act_sq = set([2, 4, 6, 1, 5, 7][:act_sq_tiles]).intersection(range(T))

    nc = bacc.Bacc("TRN2", target_bir_lowering=False, debug=False)
    x_ext = nc.dram_tensor("x", [bp, _FD], f16, kind="ExternalInput").ap()
    y_ext = nc.dram_tensor("out", [_P, T], f32, kind="ExternalOutput").ap()

    with tile.TileContext(nc) as tc, ExitStack() as ctx:
        xp = ctx.enter_context(tc.tile_pool(name="x", bufs=4))
        sp = ctx.enter_context(tc.tile_pool(name="scr", bufs=3))
        pers = ctx.enter_context(tc.tile_pool(name="pers", bufs=1))

        # wide per-tile L1 outputs: [t, d, 32] fp16
        s1w = pers.tile([_P, T * 2048], f16, tag="s1w")
        s2w = pers.tile([_P, T * 2048], f16, tag="s2w")
        p1b = pers.tile([_P, T * _D], f32, tag="p1b")
        p2b = pers.tile([_P, T * _D], f32, tag="p2b")
        sa1 = pers.tile([_P, T + 2], f32, tag="sa1")   # sin accum (cols T, T+1: tile0 pieces)
        eacc = pers.tile([_P, T], f32, tag="eacc")
        p1f = pers.tile([_P, T], f32, tag="p1f")
        dq = pers.tile([_P, T], f32, tag="dq")
        out8 = pers.tile([_P, T], f32, tag="out8")
        er = pers.tile([_P, T * _D], f32, tag="er")
        ez = pers.tile([_P, T * _D], f32, tag="ez")

        xv_dram = x_ext.rearrange("(t p) q -> t p q", p=_P)

        # warm the Sin table during the first DMA wait (lazy load is ~2.6us)
        warm = pers.tile([_P, 1], f32, tag="warm")
        nc.gpsimd.memset(warm[:], 0.0)
        nc.scalar.activation(warm[:], warm[:], AF.Sin, scale=0.125)

        def emit_heads(k, xt, nd, sacol, scol):
            """Heads for a tile buffer xt holding nd d-groups (free nd*64,
            f-halves contiguous). L1 outputs land at s1w/s2w[:, scol:]."""
            fd = nd * _F
            h = fd // 2
            xlo = xt[:, :h]
            xhi = xt[:, h:fd]
            sscr = sp.tile([_P, _FD], f16, tag="sscr")
            nc.scalar.activation(
                sscr[:, :fd], xt[:, :fd], AF.Sin, scale=0.125,
                accum_out=sa1[:, sacol:sacol + 1],
            )
            nc.vector.tensor_add(s1w[:, scol:scol + h], xlo, xhi)
            if k in act_sq:
                x2t = sp.tile([_P, _FD], f16, tag="x2")
                nc.scalar.activation(x2t[:, :fd], xt[:, :fd], AF.Square)
                nc.vector.tensor_add(
                    s2w[:, scol:scol + h], x2t[:, :h], x2t[:, h:fd]
                )
            else:
                nc.vector._custom_dve(
                    sqsum_op, out=s2w[:, scol:scol + h], in0=xlo, in1=xhi
                )

        def emit_tails(t0, t1):
            """Batched fold 32->1 for tiles [t0, t1): wide in-place fp16 2x
            adds + one fp32 final level into p1b/p2b."""
            nt = t1 - t0
            for w, dst in ((s1w, p1b), (s2w, p2b)):
                v = w[:, t0 * 2048:t1 * 2048].rearrange(
                    "p (g f) -> p g f", f=32
                )  # g = nt*64 groups
                nc.vector.tensor_add(v[:, :, :16], v[:, :, :16], v[:, :, 16:])
                nc.vector.tensor_add(v[:, :, :8], v[:, :, :8], v[:, :, 8:16])
                nc.vector.tensor_add(v[:, :, :4], v[:, :, :4], v[:, :, 4:8])
                nc.vector.tensor_add(v[:, :, :2], v[:, :, :2], v[:, :, 2:4])
                nc.vector.tensor_add(
                    dst[:, t0 * _D:t1 * _D].rearrange("p (g o) -> p g o", o=1),
                    v[:, :, 0:1], v[:, :, 1:2],
                )

        def epilogue(c0, c1):
            """Combine p1/p2/sa1 for tile-columns [c0, c1)."""
            d0, d1 = c0 * _D, c1 * _D
            nc.vector.tensor_mul(er[:, d0:d1], p1b[:, d0:d1], p1b[:, d0:d1])
            nc.vector.scalar_tensor_tensor(
                ez[:, d0:d1], p2b[:, d0:d1], 3.0, er[:, d0:d1],
                OP.mult, OP.subtract,
            )
            nc.vector.scalar_tensor_tensor(
                er[:, d0:d1], p1b[:, d0:d1], -1.0 / 6.0, ez[:, d0:d1],
                OP.mult, OP.mult,
            )
            nc.vector.reduce_sum(
                eacc[:, c0:c1],
                er[:, d0:d1].rearrange("p (t d) -> p t d", t=c1 - c0, d=_D),
                axis=AX.X,
            )
            nc.vector.reduce_sum(
                p1f[:, c0:c1],
                p1b[:, d0:d1].rearrange("p (t d) -> p t d", t=c1 - c0, d=_D),
                axis=AX.X,
            )
            # out = eacc + 128 p1f - 1024 S1   (P3 = 384 p1f - 3072 S1)
            nc.vector.scalar_tensor_tensor(
                dq[:, c0:c1], sa1[:, c0:c1], -1024.0, eacc[:, c0:c1],
                OP.mult, OP.add,
            )
            nc.vector.scalar_tensor_tensor(
                out8[:, c0:c1], p1f[:, c0:c1], 128.0, dq[:, c0:c1],
                OP.mult, OP.add,
            )

        for k in range(T):
            if k == 0:
                # two half-tiles so compute starts ~2x sooner; the host
                # layout makes each half self-contained (d-split).
                xta = sp.tile([_P, _H], f16, tag="xta")
                nc.sync.dma_start(xta[:, :_H // 2], xv_dram[0][:, :_H // 2])
                nc.sync.dma_start(xta[:, _H // 2:], xv_dram[0][:, _H // 2:_H])
                emit_heads(0, xta[:, :_H // 2], _D // 4, 0, 0)
                emit_heads(0, xta[:, _H // 2:], _D // 4, T, 512)
                xtb = sp.tile([_P, _H], f16, tag="xtb")
                nc.sync.dma_start(xtb[:], xv_dram[0][:, _H:])
                emit_heads(0, xtb, _D // 2, T + 1, 1024)
                nc.vector.scalar_tensor_tensor(
                    sa1[:, 0:1], sa1[:, T:T + 1], 1.0, sa1[:, 0:1],
                    OP.mult, OP.add,
                )
                nc.vector.scalar_tensor_tensor(
                    sa1[:, 0:1], sa1[:, T + 1:T + 2], 1.0, sa1[:, 0:1],
                    OP.mult, OP.add,
                )
            else:
                xt = xp.tile([_P, _FD], f16, tag="xt")
                nc.sync.dma_start(xt[:], xv_dram[k])
                emit_heads(k, xt, _D, k, k * 2048)
            if k == T // 2:
                emit_tails(0, T // 2)
            if k == T // 2 + 1:
                epilogue(0, T // 2)
                nc.sync.dma_start(y_ext[:, :T // 2], out8[:, :T // 2])
            if k == T - 1:
                emit_tails(T // 2, T - 1)
        emit_tails(T - 1, T)
        epilogue(T // 2, T)
        nc.sync.dma_start(y_ext[:, T // 2:], out8[:, T // 2:])

    nc.compile()
    return nc


_nc_cache = {}


def _get_nc():
    key = (_BP, _ACT_SQ_TILES)
    if key not in _nc_cache:
        _nc_cache[key] = build_nc(*key)
    return _nc_cache[key]


def _marshal(x: np.ndarray) -> list:
    """FULL fp32 input [B, F, D] -> per-core fp16 arrays [bp, 4096] in
    (tile-internal) layout: per batch, free = (h=f//32, d, f%32), with
    tile 0 of each core d-split into two self-contained halves."""
    x = np.asarray(x)
    assert x.shape == (_B, _F, _D), x.shape
    xc = x.reshape(_NCORES, _BP, _F, _D).astype(np.float16)
    xt = xc.reshape(_NCORES, _BP, 2, 32, _D).transpose(0, 1, 2, 4, 3)
    out = np.empty((_NCORES, _BP, _FD), dtype=np.float16)
    flat = xt.reshape(_NCORES, _BP, _FD)
    out[:, _P:] = flat[:, _P:]
    # tile 0 pieces: d 0-15, d 16-31 (quarters), d 32-63 (half); each piece
    # h-major so its f-halves are contiguous and self-contained
    t0 = xt[:, :_P]                                   # [c, 128, h, d, f2]
    pa = t0[:, :, :, 0:16].reshape(_NCORES, _P, 1024)
    pb = t0[:, :, :, 16:32].reshape(_NCORES, _P, 1024)
    pc = t0[:, :, :, 32:64].reshape(_NCORES, _P, 2048)
    out[:, :_P] = np.concatenate([pa, pb, pc], axis=2)
    return [np.ascontiguousarray(out[c]) for c in range(_NCORES)]


def kernel(x: np.ndarray) -> np.ndarray:
    from concourse.bass_utils import run_bass_kernel_spmd

    nc = _get_nc()
    shards = _marshal(x)
    in_maps = [{"x": shards[c]} for c in range(_NCORES)]
    res = run_bass_kernel_spmd(nc, in_maps, core_ids=list(range(_NCORES)))
    outs = []
    for c in range(_NCORES):
        o = res.results[c]["out"]  # [128, T]; o[p, t] = y[t*128 + p]
        outs.append(np.asarray(o).T.reshape(-1))
    return np.concatenate(outs).reshape(_B, 1).astype(np.float32)
